# revision 4
# baseline (speedup 1.0000x reference)
"""Trainium2 Bass kernel for AutoRegressiveLSTMEncoder — v2.

Strategy: pure data parallel over 8 NeuronCores (batch 32768 -> 4096/core).
Feature-on-partition / batch-on-free layout; every matmul is lhsT.T @ rhs
with K on partitions.

v2 changes vs baseline:
  - FULL UNROLL of the 32 LSTM steps (no tc.For_i): removes the per-iteration
    InstAllEngineBarrier + semaphore reset + swdge dynamic-DMA descriptors,
    enabling cross-step software pipelining. Instruction stream is fetched
    autonomously from HBM, so IRAM size is not a constraint.
  - h and p state are SBUF-RESIDENT (bf16, updated in place per batch tile):
    no HBM round-trip for the recurrent state.
  - c state streams through HBM (fp32, ping-pong) — it only feeds elementwise
    ops, so latency is hidden by the n-tile pipeline.
  - Step 0 specialized: h=c=p=0, so gates = act(G0) directly (no matmuls).
  - Prologue fused: t_h tiles stay in SBUF per batch tile; G0 written to HBM
    once (read back every step; 32MB/step @ 358GB/s hides under PE work).
  - Output p_all[t] indexed statically: [D, A, BL] fp32.

Algebraic optimizations kept from baseline:
  - softmax(log(softplus(s)+eps)) == (softplus(s)+eps)/sum(...) — and the
    +eps is dropped (softplus >= ~0.6 here, eps=1e-6 is far below tolerance).
  - W_big = W_ih[:, H:] @ W_emb folded once on host.
  - G0 = W_ih[:, :H] @ t_h + b_ih + b_hh precomputed (step-invariant).
"""

import sys

sys.path.insert(0, "/opt/trn_rl_repo")

import numpy as np
import ml_dtypes
from contextlib import ExitStack

import concourse.bass as bass
import concourse.bacc as bacc
import concourse.tile as tile
from concourse import mybir

AF = mybir.ActivationFunctionType
DT = mybir.dt

B, E, D, A, H = 32768, 300, 32, 64, 1024
G4 = 4 * H
NCORES = 8
BL = B // NCORES  # 4096
NT = 512
NB = BL // NT  # 8
KXP = 384  # E=300 padded to 3*128


def build_nc(BL=BL, nsteps=D):
    NB = BL // NT
    assert BL == NB * NT

    nc = bacc.Bacc("TRN2", target_bir_lowering=False, debug=False)
    f32, bf = DT.float32, DT.bfloat16

    # ---- external inputs (host pre-tiled / pre-transposed / pre-cast) ----
    xT = nc.dram_tensor("xT", (3, 128, BL), bf, kind="ExternalInput")
    WxhT = nc.dram_tensor("WxhT", (3, 128, H), bf, kind="ExternalInput")
    bxh = nc.dram_tensor("bxh", (128, 8), f32, kind="ExternalInput")
    WihAT = nc.dram_tensor("WihAT", (8, 128, G4), bf, kind="ExternalInput")
    WbigT = nc.dram_tensor("WbigT", (A, G4), bf, kind="ExternalInput")
    WhhT = nc.dram_tensor("WhhT", (8, 128, G4), bf, kind="ExternalInput")
    bg = nc.dram_tensor("bg", (128, 32), f32, kind="ExternalInput")
    WhzT = nc.dram_tensor("WhzT", (8, 128, A), bf, kind="ExternalInput")
    bhz = nc.dram_tensor("bhz", (A, 1), f32, kind="ExternalInput")
    onesA = nc.dram_tensor("onesA", (A, 1), f32, kind="ExternalInput")
    ones1 = nc.dram_tensor("ones1", (1, 128), f32, kind="ExternalInput")

    # ---- output ----
    p_all = nc.dram_tensor("p_all", (nsteps, A, BL), f32, kind="ExternalOutput")

    # ---- internal DRAM scratch ----
    G0_d = nc.dram_tensor("G0_d", (NB, 8, 128, 4 * NT), bf, kind="Internal")
    c_d = [
        nc.dram_tensor(f"c_d{i}", (NB, 8, 128, NT), f32, kind="Internal")
        for i in (0, 1)
    ]

    with tile.TileContext(nc) as tc, ExitStack() as ctx:
        # ========== prologue: t_h and G0 (fused per batch tile) ==========
        # Prologue pools are released before the resident pool is allocated,
        # so their SBUF (wa alone is 64KB/p) is recycled for h/weights.
        with ExitStack() as pro:
            wpro = pro.enter_context(tc.tile_pool(name="wpro", bufs=1))
            wxh = [wpro.tile([128, H], bf, tag=f"wxh{k}", name=f"wxh{k}") for k in range(3)]
            wa = [wpro.tile([128, G4], bf, tag=f"wa{k}", name=f"wa{k}") for k in range(8)]
            bxh_t = wpro.tile([128, 8], f32, tag="bxh")
            bg_t = wpro.tile([128, 32], f32, tag="bg")
            for k in range(3):
                nc.sync.dma_start(wxh[k][:], WxhT[k])
            for k in range(8):
                nc.sync.dma_start(wa[k][:], WihAT[k])
            nc.sync.dma_start(bxh_t[:], bxh[:])
            nc.sync.dma_start(bg_t[:], bg[:])

            pps = pro.enter_context(tc.tile_pool(name="pps", bufs=8, space="PSUM"))
            xr_p = pro.enter_context(tc.tile_pool(name="pxr", bufs=2))
            th_p = pro.enter_context(tc.tile_pool(name="pth", bufs=2))
            g0_p = pro.enter_context(tc.tile_pool(name="pg0", bufs=2))
            for n in range(NB):
                sl = slice(n * NT, (n + 1) * NT)
                xr = [xr_p.tile([128, NT], bf, tag=f"xr{k}", name=f"xr{k}") for k in range(3)]
                for k in range(3):
                    nc.sync.dma_start(xr[k][:], xT[k][:, sl])
                th = [th_p.tile([128, NT], bf, tag=f"th{k}", name=f"th{k}") for k in range(8)]
                for mh in range(8):
                    ps = pps.tile([128, NT], f32, tag="ps")
                    for k in range(3):
                        nc.tensor.matmul(
                            ps[:],
                            wxh[k][:, mh * 128 : (mh + 1) * 128],
                            xr[k][:],
                            start=(k == 0),
                            stop=(k == 2),
                        )
                    nc.scalar.activation(th[mh][:], ps[:], AF.Tanh, bias=bxh_t[:, mh : mh + 1])
                for r in range(8):
                    g0t = g0_p.tile([128, 4 * NT], bf, tag="g0t")
                    for gi in range(4):
                        m = gi * 8 + r
                        ps = pps.tile([128, NT], f32, tag="ps")
                        for k in range(8):
                            nc.tensor.matmul(
                                ps[:],
                                wa[k][:, m * 128 : (m + 1) * 128],
                                th[k][:],
                                start=(k == 0),
                                stop=(k == 7),
                            )
                        nc.scalar.activation(
                            g0t[:, gi * NT : (gi + 1) * NT],
                            ps[:],
                            AF.Identity,
                            bias=bg_t[:, m : m + 1],
                        )
                    nc.sync.dma_start(G0_d[n, r], g0t[:])

        # ========== resident weights + state (live after prologue) ==========
        wres = ctx.enter_context(tc.tile_pool(name="wres", bufs=1))
        wh = [wres.tile([128, G4], bf, tag=f"wh{k}", name=f"wh{k}") for k in range(8)]
        wb_t = wres.tile([A, G4], bf, tag="wbig")
        wz = [wres.tile([128, A], bf, tag=f"wz{k}", name=f"wz{k}") for k in range(8)]
        bhz_t = wres.tile([A, 1], f32, tag="bhz")
        onesA_t = wres.tile([A, 1], f32, tag="onesA")
        ones1_t = wres.tile([1, 128], f32, tag="ones1")
        for k in range(8):
            nc.sync.dma_start(wh[k][:], WhhT[k])
        nc.sync.dma_start(wb_t[:], WbigT[:])
        for k in range(8):
            nc.sync.dma_start(wz[k][:], WhzT[k])
        nc.sync.dma_start(bhz_t[:], bhz[:])
        nc.sync.dma_start(onesA_t[:], onesA[:])
        nc.sync.dma_start(ones1_t[:], ones1[:])

        # resident recurrent state (written fully at t=0 before any read)
        h_res = [wres.tile([128, BL], bf, tag=f"h{k}", name=f"h{k}") for k in range(8)]
        p_res = wres.tile([A, BL], bf, tag="pres")

        # ========== main loop pools ==========
        psum = ctx.enter_context(tc.tile_pool(name="psum", bufs=5, space="PSUM"))
        psz = ctx.enter_context(tc.tile_pool(name="psz", bufs=1, space="PSUM"))
        g0r_p = ctx.enter_context(tc.tile_pool(name="g0r", bufs=3))
        cin_p = ctx.enter_context(tc.tile_pool(name="cin", bufs=3))
        cell_p = ctx.enter_context(tc.tile_pool(name="cell", bufs=2))
        zp_p = ctx.enter_context(tc.tile_pool(name="zp", bufs=2))

        for t in range(nsteps):
            rb, wb = t % 2, (t + 1) % 2
            for n in range(NB):
                sl = slice(n * NT, (n + 1) * NT)
                for r in range(8):
                    g0t = g0r_p.tile([128, 4 * NT], bf, tag="g0t")
                    nc.sync.dma_start(g0t[:], G0_d[n, r])
                    if t > 0:
                        cin = cin_p.tile([128, NT], f32, tag="cin")
                        nc.sync.dma_start(cin[:], c_d[rb][n, r])
                    gates = []
                    for gi in range(4):
                        m = gi * 8 + r
                        gsb = cell_p.tile(
                            [128, NT], bf, tag=f"gate{gi}", name=f"gate{gi}"
                        )
                        act_f = AF.Tanh if gi == 2 else AF.Sigmoid
                        if t == 0:
                            # h=c=p=0: gates come straight from G0
                            nc.scalar.activation(
                                gsb[:], g0t[:, gi * NT : (gi + 1) * NT], act_f
                            )
                        else:
                            ps = psum.tile([128, NT], f32, tag="ps")
                            for k in range(8):
                                nc.tensor.matmul(
                                    ps[:],
                                    wh[k][:, m * 128 : (m + 1) * 128],
                                    h_res[k][:, sl],
                                    start=(k == 0),
                                    stop=False,
                                )
                            nc.tensor.matmul(
                                ps[:],
                                wb_t[:, m * 128 : (m + 1) * 128],
                                p_res[:, sl],
                                start=False,
                                stop=True,
                            )
                            nc.vector.tensor_tensor(
                                ps[:], ps[:], g0t[:, gi * NT : (gi + 1) * NT],
                                mybir.AluOpType.add,
                            )
                            nc.scalar.activation(gsb[:], ps[:], act_f)
                        gates.append(gsb)
                    i_sb, f_sb, g_sb, o_sb = gates
                    # c' = f*c + i*g ; h = o*tanh(c')
                    cnew = cell_p.tile([128, NT], f32, tag="cnew")
                    if t == 0:
                        nc.vector.tensor_tensor(
                            cnew[:], g_sb[:], i_sb[:], mybir.AluOpType.mult
                        )
                    else:
                        ig = cell_p.tile([128, NT], bf, tag="ig")
                        nc.vector.tensor_tensor(
                            ig[:], g_sb[:], i_sb[:], mybir.AluOpType.mult
                        )
                        fc = cell_p.tile([128, NT], f32, tag="fc")
                        nc.vector.tensor_tensor(
                            fc[:], f_sb[:], cin[:], mybir.AluOpType.mult
                        )
                        nc.vector.tensor_tensor(
                            cnew[:], fc[:], ig[:], mybir.AluOpType.add
                        )
                    if t < nsteps - 1:
                        nc.sync.dma_start(c_d[wb][n, r], cnew[:])
                    tht = cell_p.tile([128, NT], f32, tag="tht")
                    nc.scalar.activation(tht[:], cnew[:], AF.Tanh)
                    nc.vector.tensor_tensor(
                        h_res[r][:, sl], o_sb[:], tht[:], mybir.AluOpType.mult
                    )
                # ---- z/p phase (uses the just-written h(t+1)) ----
                zps = psz.tile([A, NT], f32, tag="zps")
                for k in range(8):
                    nc.tensor.matmul(
                        zps[:], wz[k][:], h_res[k][:, sl], start=(k == 0), stop=(k == 7)
                    )
                u = zp_p.tile([A, NT], f32, tag="u")
                nc.scalar.activation(u[:], zps[:], AF.Exp, bias=bhz_t[:])
                q2 = zp_p.tile([A, NT], f32, tag="q2")
                nc.scalar.activation(q2[:], u[:], AF.Ln, bias=onesA_t[:])
                sps = psz.tile([1, NT], f32, tag="sps")
                nc.tensor.matmul(sps[:], onesA_t[:], q2[:], start=True, stop=True)
                rec = zp_p.tile([1, NT], f32, tag="rec")
                nc.vector.reciprocal(rec[:], sps[:])
                rbc = psz.tile([128, NT], f32, tag="rbc")
                nc.tensor.matmul(rbc[:], ones1_t[:], rec[:], start=True, stop=True)
                pt = zp_p.tile([A, NT], f32, tag="pt")
                nc.vector.tensor_tensor(pt[:], q2[:], rbc[:A, :], mybir.AluOpType.mult)
                nc.sync.dma_start(p_all[t][:, sl], pt[:])
                if t < nsteps - 1:
                    nc.vector.tensor_copy(p_res[:, sl], pt[:])

    nc.compile()
    return nc


# ---------------- host-side wrapper ----------------


def _prep_weights(W_xh, b_xh, W_ih, W_hh, b_ih, b_hh, W_hz, b_hz, W_emb):
    bf = ml_dtypes.bfloat16
    f32 = np.float32
    d = {}
    wxh = np.zeros((KXP, H), f32)
    wxh[:E] = np.asarray(W_xh, f32).T
    d["WxhT"] = np.ascontiguousarray(wxh.reshape(3, 128, H)).astype(bf)
    d["bxh"] = np.ascontiguousarray(np.asarray(b_xh, f32).reshape(8, 128).T)
    wih = np.asarray(W_ih, f32)
    d["WihAT"] = np.ascontiguousarray(wih[:, :H].T.reshape(8, 128, G4)).astype(bf)
    wbig = wih[:, H:].astype(np.float64) @ np.asarray(W_emb, np.float64)
    d["WbigT"] = np.ascontiguousarray(wbig.T.astype(np.float32)).astype(bf)
    d["WhhT"] = np.ascontiguousarray(np.asarray(W_hh, f32).T.reshape(8, 128, G4)).astype(bf)
    d["bg"] = np.ascontiguousarray(
        (np.asarray(b_ih, f32) + np.asarray(b_hh, f32)).reshape(32, 128).T
    )
    d["WhzT"] = np.ascontiguousarray(np.asarray(W_hz, f32).T.reshape(8, 128, A)).astype(bf)
    d["bhz"] = np.ascontiguousarray(np.asarray(b_hz, f32).reshape(A, 1))
    d["onesA"] = np.ones((A, 1), f32)
    d["ones1"] = np.ones((1, 128), f32)
    return d


def _prep_x(x_shard):
    bf = ml_dtypes.bfloat16
    xt = np.zeros((KXP, x_shard.shape[0]), np.float32)
    xt[:E] = np.asarray(x_shard, np.float32).T
    return np.ascontiguousarray(xt.reshape(3, 128, -1)).astype(bf)


def build_for_timing(inputs, **build_kwargs):
    wd = _prep_weights(**{k: v for k, v in inputs.items() if k != "input_x"})
    x = np.asarray(inputs["input_x"], np.float32)
    in_maps = []
    for c in range(NCORES):
        m = dict(wd)
        m["xT"] = _prep_x(x[c * BL : (c + 1) * BL])
        in_maps.append(m)
    return build_nc(**build_kwargs), in_maps


def kernel(input_x, W_xh, b_xh, W_ih, W_hh, b_ih, b_hh, W_hz, b_hz, W_emb):
    from concourse.bass_utils import run_bass_kernel_spmd

    wd = _prep_weights(W_xh, b_xh, W_ih, W_hh, b_ih, b_hh, W_hz, b_hz, W_emb)
    x = np.asarray(input_x, np.float32)
    in_maps = []
    for c in range(NCORES):
        m = dict(wd)
        m["xT"] = _prep_x(x[c * BL : (c + 1) * BL])
        in_maps.append(m)

    nc = build_nc()
    res = run_bass_kernel_spmd(nc, in_maps, list(range(NCORES)))

    out = np.empty((B, D, A), np.float32)
    for c in range(NCORES):
        pa = res.results[c]["p_all"]  # [D, A, BL]
        out[c * BL : (c + 1) * BL] = pa.transpose(2, 0, 1)
    return out, out


# revision 5
# speedup vs baseline: 1.0077x; 1.0077x over previous
"""Trainium2 Bass kernel for AutoRegressiveLSTMEncoder — v4.

Data parallel over 8 NeuronCores (batch 32768 -> 4096/core); feature-on-
partition / batch-on-free layout; full unroll of the 32 steps; h/p state
SBUF-resident; c streamed bf16 through HBM (ping-pong).

Engine-balance design (per step, per core, ~224us PE / ~170us ACT / ~130us
DVE / ~137us DMA):
  - Recurrent matmuls in fp8e4 DoubleRow (2 weights/PE-cell, K=256/matmul).
  - G0 (the step-invariant input term) enters PSUM via an identity matmul,
    not a vector add: PE absorbs it, DVE stays light.
  - ALL gate activations are Sigmoid: tanh(x)=2*sigmoid(2x)-1, with the *2
    folded into the g-gate weight rows on the host (row scale 128 vs 64) and
    the affine fix done by one DVE tensor_scalar. One activation-table swap
    pair per STEP (sigmoid <-> exp/ln), not per tile.
  - The 4 gate blocks accumulate into one [128, 4*512] PSUM tile (4 banks),
    evicted by a single wide Sigmoid activation (amortizes the ~185ns ACT
    fixed cost). One PSUM pool: 2 x 4-bank slots; the deferred z-phase
    cycles through the same ring.
  - z-phase: softmax denominator computed as ones[64x128].T @ q2 -> [128,NT]
    (sum replicated across partitions in one matmul; no separate broadcast).
  - Step 0 specialized (h=c=p=0 -> gates straight from G0).

Numerics: weights carry a x64 scale (x128 for the g-gate) folded on the host;
activations descale with scale=1/64. fp8 noise enters only via the recurrent
term (the dominant G0 term stays bf16).
"""

import sys

sys.path.insert(0, "/opt/trn_rl_repo")

import numpy as np
import ml_dtypes
from contextlib import ExitStack

import concourse.bass as bass
import concourse.bacc as bacc
import concourse.tile as tile
from concourse import mybir

AF = mybir.ActivationFunctionType
DT = mybir.dt
ALU = mybir.AluOpType

B, E, D, A, H = 32768, 300, 32, 64, 1024
G4 = 4 * H
NCORES = 8
BL = B // NCORES  # 4096
NT = 512
NB = BL // NT  # 8
KXP = 384  # E=300 padded to 3*128
SCALE = 64.0


def build_nc(BL=BL, nsteps=D):
    NB = BL // NT
    assert BL == NB * NT

    nc = bacc.Bacc("TRN2", target_bir_lowering=False, debug=False)
    f32, bf, f8 = DT.float32, DT.bfloat16, DT.float8e4
    DR = mybir.MatmulPerfMode.DoubleRow

    # ---- external inputs (host pre-tiled / pre-transposed / pre-cast) ----
    xT = nc.dram_tensor("xT", (3, 128, BL), bf, kind="ExternalInput")
    WxhT = nc.dram_tensor("WxhT", (3, 128, H), bf, kind="ExternalInput")
    bxh = nc.dram_tensor("bxh", (128, 8), f32, kind="ExternalInput")
    WihADR = nc.dram_tensor("WihADR", (4, 128, 2, G4), f8, kind="ExternalInput")
    WbigT = nc.dram_tensor("WbigT", (A, G4), bf, kind="ExternalInput")
    WhhDR = nc.dram_tensor("WhhDR", (4, 128, 2, G4), f8, kind="ExternalInput")
    bg = nc.dram_tensor("bg", (128, 32), f32, kind="ExternalInput")
    WhzDR = nc.dram_tensor("WhzDR", (4, 128, 2, A), f8, kind="ExternalInput")
    ident = nc.dram_tensor("ident", (128, 128), bf, kind="ExternalInput")
    bhz = nc.dram_tensor("bhz", (A, 1), f32, kind="ExternalInput")
    onesA = nc.dram_tensor("onesA", (A, 1), f32, kind="ExternalInput")
    onesB = nc.dram_tensor("onesB", (A, 128), f32, kind="ExternalInput")

    # ---- output ----
    p_all = nc.dram_tensor("p_all", (nsteps, A, BL), f32, kind="ExternalOutput")

    # ---- internal DRAM scratch ----
    G0_d = nc.dram_tensor("G0_d", (NB, 8, 128, 4 * NT), bf, kind="Internal")
    c_d = [
        nc.dram_tensor(f"c_d{i}", (NB, 8, 128, NT), bf, kind="Internal")
        for i in (0, 1)
    ]

    with tile.TileContext(nc) as tc, ExitStack() as ctx:
        # ========== prologue: t_h and G0 (fused per batch tile) ==========
        with ExitStack() as pro:
            wpro = pro.enter_context(tc.tile_pool(name="wpro", bufs=1))
            wxh = [wpro.tile([128, H], bf, tag=f"wxh{k}", name=f"wxh{k}") for k in range(3)]
            wa = [
                wpro.tile([128, 2, G4], f8, tag=f"wa{k}", name=f"wa{k}")
                for k in range(4)
            ]
            bxh_t = wpro.tile([128, 8], f32, tag="bxh")
            bg_t = wpro.tile([128, 32], f32, tag="bg")
            for k in range(3):
                nc.sync.dma_start(wxh[k][:], WxhT[k])
            for k in range(4):
                nc.sync.dma_start(wa[k][:], WihADR[k])
            nc.sync.dma_start(bxh_t[:], bxh[:])
            nc.sync.dma_start(bg_t[:], bg[:])

            pps = pro.enter_context(tc.tile_pool(name="pps", bufs=8, space="PSUM"))
            xr_p = pro.enter_context(tc.tile_pool(name="pxr", bufs=2))
            th_p = pro.enter_context(tc.tile_pool(name="pth", bufs=2))
            g0_p = pro.enter_context(tc.tile_pool(name="pg0", bufs=2))
            for n in range(NB):
                sl = slice(n * NT, (n + 1) * NT)
                xr = [xr_p.tile([128, NT], bf, tag=f"xr{k}", name=f"xr{k}") for k in range(3)]
                for k in range(3):
                    nc.sync.dma_start(xr[k][:], xT[k][:, sl])
                # t_h stored fp8, DoubleRow-interleaved (rhs for the G0 matmuls)
                thdr = [
                    th_p.tile([128, 2, NT], f8, tag=f"th{k}", name=f"th{k}")
                    for k in range(4)
                ]
                for mh in range(8):
                    ps = pps.tile([128, NT], f32, tag="ps")
                    for k in range(3):
                        nc.tensor.matmul(
                            ps[:],
                            wxh[k][:, mh * 128 : (mh + 1) * 128],
                            xr[k][:],
                            start=(k == 0),
                            stop=(k == 2),
                        )
                    nc.scalar.activation(
                        thdr[mh // 2][:, mh % 2, :], ps[:], AF.Tanh,
                        bias=bxh_t[:, mh : mh + 1],
                    )
                for r in range(8):
                    g0t = g0_p.tile([128, 4 * NT], bf, tag="g0t")
                    for gi in range(4):
                        m = gi * 8 + r
                        ps = pps.tile([128, NT], f32, tag="ps")
                        for k in range(4):
                            nc.tensor.matmul(
                                ps[:],
                                wa[k][:, :, m * 128 : (m + 1) * 128],
                                thdr[k][:],
                                start=(k == 0),
                                stop=(k == 3),
                                perf_mode=DR,
                            )
                        nc.scalar.activation(
                            g0t[:, gi * NT : (gi + 1) * NT],
                            ps[:],
                            AF.Identity,
                            bias=bg_t[:, m : m + 1],
                        )
                    nc.sync.dma_start(G0_d[n, r], g0t[:])

        # ========== resident weights + state (live after prologue) ==========
        wres = ctx.enter_context(tc.tile_pool(name="wres", bufs=1))
        whdr = [
            wres.tile([128, 2, G4], f8, tag=f"wh{k}", name=f"wh{k}") for k in range(4)
        ]
        wb_t = wres.tile([A, G4], bf, tag="wbig")
        wzdr = [
            wres.tile([128, 2, A], f8, tag=f"wz{k}", name=f"wz{k}") for k in range(4)
        ]
        id_t = wres.tile([128, 128], bf, tag="ident")
        bhz_t = wres.tile([A, 1], f32, tag="bhz")
        onesA_t = wres.tile([A, 1], f32, tag="onesA")
        onesB_t = wres.tile([A, 128], f32, tag="onesB")
        for k in range(4):
            nc.sync.dma_start(whdr[k][:], WhhDR[k])
        nc.sync.dma_start(wb_t[:], WbigT[:])
        for k in range(4):
            nc.sync.dma_start(wzdr[k][:], WhzDR[k])
        nc.sync.dma_start(id_t[:], ident[:])
        nc.sync.dma_start(bhz_t[:], bhz[:])
        nc.sync.dma_start(onesA_t[:], onesA[:])
        nc.sync.dma_start(onesB_t[:], onesB[:])

        # resident recurrent state (written fully at t=0 before any read);
        # h lives DoubleRow-interleaved: hdr[P][kp, j, b] = h[(2P+j)*128+kp, b]
        hdr = [
            wres.tile([128, 2, BL], f8, tag=f"h{k}", name=f"h{k}") for k in range(4)
        ]
        p_res = wres.tile([A, BL], bf, tag="pres")

        # ========== main loop pools ==========
        # gates: 6 one-bank slots; z: 2 one-bank slots (own pool so the
        # deferred z chains never block the next step's gate matmuls)
        psum = ctx.enter_context(tc.tile_pool(name="psum", bufs=6, space="PSUM"))
        psz = ctx.enter_context(tc.tile_pool(name="psz", bufs=2, space="PSUM"))
        g0r_p = ctx.enter_context(tc.tile_pool(name="g0r", bufs=4))
        cin_p = ctx.enter_context(tc.tile_pool(name="cin", bufs=4))
        cell_p = ctx.enter_context(tc.tile_pool(name="cell", bufs=4))
        zp_p = ctx.enter_context(tc.tile_pool(name="zp", bufs=4))

        for t in range(nsteps):
            rb, wb = t % 2, (t + 1) % 2
            for n in range(NB):
                sl = slice(n * NT, (n + 1) * NT)
                # hoist the whole tile's loads ahead of any dependent store in
                # the (in-order) SP DMA queue so prefetch never stalls the PE
                g0ts, cins = [], []
                for r in range(8):
                    g0t = g0r_p.tile([128, 4 * NT], bf, tag="g0t")
                    nc.sync.dma_start(g0t[:], G0_d[n, r])
                    g0ts.append(g0t)
                    if t > 0:
                        cin = cin_p.tile([128, NT], bf, tag="cin")
                        nc.sync.dma_start(cin[:], c_d[rb][n, r])
                        cins.append(cin)
                # tail of iteration r-1 (csig/cm/h) is emitted after the gate
                # activations of iteration r: the ACT stream then never stalls
                # on the DVE chain (csig's input is ready an iteration early),
                # and the final two elementwise ops run on the idle GpSimd.
                def emit_tail(prev):
                    cnew_p, o_p, r_p = prev
                    csig = cell_p.tile([128, NT], bf, tag="csig")
                    nc.scalar.activation(csig[:], cnew_p[:], AF.Sigmoid, scale=2.0)
                    cm = cell_p.tile([128, NT], bf, tag="cm")
                    nc.gpsimd.tensor_scalar(cm[:], csig[:], 0.5, None, ALU.subtract)
                    nc.gpsimd.tensor_tensor(
                        hdr[r_p // 2][:, r_p % 2, sl], cm[:], o_p, ALU.mult
                    )

                prev = None
                for r in range(8):
                    g0t = g0ts[r]
                    if t > 0:
                        cin = cins[r]
                    # gate blocks: one PSUM bank each, evicted by sigmoid ACT
                    # into slices of one SBUF tile
                    gs4 = cell_p.tile([128, 4 * NT], bf, tag="gs4")
                    if t == 0:
                        nc.scalar.activation(
                            gs4[:], g0t[:], AF.Sigmoid, scale=1.0 / SCALE
                        )
                    else:
                        for gi in range(4):
                            m = gi * 8 + r
                            pss = psum.tile([128, NT], f32, tag="ps")
                            nc.tensor.matmul(
                                pss[:],
                                id_t[:],
                                g0t[:, gi * NT : (gi + 1) * NT],
                                start=True,
                                stop=False,
                            )
                            for k in range(4):
                                nc.tensor.matmul(
                                    pss[:],
                                    whdr[k][:, :, m * 128 : (m + 1) * 128],
                                    hdr[k][:, :, sl],
                                    start=False,
                                    stop=False,
                                    perf_mode=DR,
                                )
                            nc.tensor.matmul(
                                pss[:],
                                wb_t[:, m * 128 : (m + 1) * 128],
                                p_res[:, sl],
                                start=False,
                                stop=True,
                            )
                            nc.scalar.activation(
                                gs4[:, gi * NT : (gi + 1) * NT], pss[:],
                                AF.Sigmoid, scale=1.0 / SCALE,
                            )
                    if prev is not None:
                        emit_tail(prev)
                    i_sb = gs4[:, 0:NT]
                    f_sb = gs4[:, NT : 2 * NT]
                    graw = gs4[:, 2 * NT : 3 * NT]  # = sigmoid(2x), tanh(x)=2*graw-1
                    o_sb = gs4[:, 3 * NT : 4 * NT]
                    # plain ts/tt ops run at DVE 2x/4x rates (the fused
                    # scalar_tensor_tensor runs at 1x only)
                    gm = cell_p.tile([128, NT], bf, tag="gm")
                    nc.vector.tensor_scalar(
                        gm[:], graw, 0.5, None, ALU.subtract
                    )  # = g/2
                    tg = cell_p.tile([128, NT], bf, tag="tg")
                    nc.vector.tensor_tensor(tg[:], gm[:], i_sb, ALU.mult)  # = i*g/2
                    cnew = cell_p.tile([128, NT], bf, tag="cnew")
                    if t == 0:
                        nc.vector.tensor_scalar(
                            cnew[:], tg[:], 2.0, None, ALU.mult
                        )
                    else:
                        fc = cell_p.tile([128, NT], bf, tag="fc")
                        nc.vector.tensor_tensor(fc[:], f_sb, cin[:], ALU.mult)
                        # c' = f*c + 2*(i*g/2)
                        ig = cell_p.tile([128, NT], bf, tag="ig")
                        nc.vector.tensor_scalar(ig[:], tg[:], 2.0, None, ALU.mult)
                        nc.vector.tensor_tensor(cnew[:], fc[:], ig[:], ALU.add)
                    if t < nsteps - 1:
                        nc.sync.dma_start(c_d[wb][n, r], cnew[:])
                    prev = (cnew, o_sb, r)
                emit_tail(prev)
                # ---- z/p phase (interleaved per tile so the next step's
                # gates, which need p_res[n], never wait on a step-end convoy)
                zps = psz.tile([A, NT], f32, tag="zz")
                for k in range(4):
                    nc.tensor.matmul(
                        zps[:], wzdr[k][:], hdr[k][:, :, sl],
                        start=(k == 0), stop=(k == 3), perf_mode=DR,
                    )
                u = zp_p.tile([A, NT], f32, tag="u")
                nc.scalar.activation(
                    u[:], zps[:], AF.Exp, bias=bhz_t[:], scale=1.0 / SCALE
                )
                q2 = zp_p.tile([A, NT], f32, tag="q2")
                nc.scalar.activation(q2[:], u[:], AF.Ln, bias=onesA_t[:])
                # sum over A, replicated across all 128 partitions in one mm
                sb = psz.tile([128, NT], f32, tag="zz")
                nc.tensor.matmul(sb[:], onesB_t[:], q2[:], start=True, stop=True)
                rec = zp_p.tile([A, NT], f32, tag="rec")
                nc.vector.reciprocal(rec[:], sb[:A, :])
                pt = zp_p.tile([A, NT], f32, tag="pt")
                nc.vector.tensor_tensor(pt[:], q2[:], rec[:], ALU.mult)
                nc.sync.dma_start(p_all[t][:, sl], pt[:])
                if t < nsteps - 1:
                    nc.vector.tensor_copy(p_res[:, sl], pt[:])

    nc.compile()
    return nc


# ---------------- host-side wrapper ----------------


def _prep_weights(W_xh, b_xh, W_ih, W_hh, b_ih, b_hh, W_hz, b_hz, W_emb):
    bf = ml_dtypes.bfloat16
    f8 = ml_dtypes.float8_e4m3
    f32 = np.float32
    d = {}
    # per-gate-row scale: x64 for i,f,o; x128 for g (tanh-via-sigmoid).
    # h is stored as h/2, so W_hh and W_hz carry an extra x2.
    rs = np.concatenate([
        np.full(H, SCALE, f32), np.full(H, SCALE, f32),
        np.full(H, 2 * SCALE, f32), np.full(H, SCALE, f32),
    ])  # [4H]
    wxh = np.zeros((KXP, H), f32)
    wxh[:E] = np.asarray(W_xh, f32).T
    d["WxhT"] = np.ascontiguousarray(wxh.reshape(3, 128, H)).astype(bf)
    d["bxh"] = np.ascontiguousarray(np.asarray(b_xh, f32).reshape(8, 128).T)
    wih = np.asarray(W_ih, f32)
    wa8 = (wih[:, :H].T * rs).reshape(4, 2, 128, G4)
    d["WihADR"] = np.ascontiguousarray(wa8.transpose(0, 2, 1, 3)).astype(f8)
    wbig = wih[:, H:].astype(np.float64) @ np.asarray(W_emb, np.float64)
    d["WbigT"] = np.ascontiguousarray((wbig.T * rs).astype(np.float32)).astype(bf)
    whh8 = (np.asarray(W_hh, f32).T * (2 * rs)).reshape(4, 2, 128, G4)
    d["WhhDR"] = np.ascontiguousarray(whh8.transpose(0, 2, 1, 3)).astype(f8)
    d["bg"] = np.ascontiguousarray(
        ((np.asarray(b_ih, f32) + np.asarray(b_hh, f32)) * rs).reshape(32, 128).T
    )
    whz8 = (np.asarray(W_hz, f32).T * (2 * SCALE)).reshape(4, 2, 128, A)
    d["WhzDR"] = np.ascontiguousarray(whz8.transpose(0, 2, 1, 3)).astype(f8)
    d["bhz"] = np.ascontiguousarray(np.asarray(b_hz, f32).reshape(A, 1))
    d["onesA"] = np.ones((A, 1), f32)
    d["onesB"] = np.ones((A, 128), f32)
    d["ident"] = np.eye(128, dtype=f32).astype(bf)
    return d


def _prep_x(x_shard):
    bf = ml_dtypes.bfloat16
    xt = np.zeros((KXP, x_shard.shape[0]), np.float32)
    xt[:E] = np.asarray(x_shard, np.float32).T
    return np.ascontiguousarray(xt.reshape(3, 128, -1)).astype(bf)


def build_for_timing(inputs, **build_kwargs):
    wd = _prep_weights(**{k: v for k, v in inputs.items() if k != "input_x"})
    x = np.asarray(inputs["input_x"], np.float32)
    in_maps = []
    for c in range(NCORES):
        m = dict(wd)
        m["xT"] = _prep_x(x[c * BL : (c + 1) * BL])
        in_maps.append(m)
    return build_nc(**build_kwargs), in_maps


def kernel(input_x, W_xh, b_xh, W_ih, W_hh, b_ih, b_hh, W_hz, b_hz, W_emb):
    from concourse.bass_utils import run_bass_kernel_spmd

    wd = _prep_weights(W_xh, b_xh, W_ih, W_hh, b_ih, b_hh, W_hz, b_hz, W_emb)
    x = np.asarray(input_x, np.float32)
    in_maps = []
    for c in range(NCORES):
        m = dict(wd)
        m["xT"] = _prep_x(x[c * BL : (c + 1) * BL])
        in_maps.append(m)

    nc = build_nc()
    res = run_bass_kernel_spmd(nc, in_maps, list(range(NCORES)))

    out = np.empty((B, D, A), np.float32)
    for c in range(NCORES):
        pa = res.results[c]["p_all"]  # [D, A, BL]
        out[c * BL : (c + 1) * BL] = pa.transpose(2, 0, 1)
    return out, out
